# revision 25
# baseline (speedup 1.0000x reference)
"""MTLU (histogram-binning piecewise-linear unit) Trainium2 kernel, v2.

Math: out = w[c,idx]*x + b[c,idx], idx = clip(floor(x/0.1)+10, 0, 19).
Continuous PWL with uniform breakpoints t_k=(k-10)/10, k=1..19:
    out = w0[c]*x + b0[c] + sum_k d_k[c]*relu(x - t_k).

v2 spreads the 19 relu terms over FOUR engines:
  ACT   J-deep composed-Prelu chain (terms S_ACT, fp32) -> h
  DVE   TSP relu features at 4x bf16 (terms F_D) + PAIRT pair chain
        (terms PAIRS k,k+10) + the affine correction (first PAIRT,
        imm2=-9 trick: relu(x+9),relu(x+8) always active)
  Pool  TSP relu features (terms F_P, GPSIMD)
  PE    diagonal matmuls accumulate d_k*feat_k + 1*h into PSUM (bf16
        moving operands, per-channel d_k on the diagonal)
The DVE PAIRT chain is seeded with Src1 = PSUM (reads the PE total),
adds the affine + pair terms, writes the fp32 output.

Sharding: pure data parallel over batch - 2 batches/core x 8 cores.
Layout [2*64, 65536]: channel on partitions, coefficients per-partition.
"""

import sys

import numpy as np

try:  # concourse is normally on sys.path via sitecustomize
    import concourse  # noqa: F401
except ImportError:  # pragma: no cover
    for _p in ("/opt/trn_rl_repo", "/root/.axon_site/_ro/trn_rl_repo"):
        if _p not in sys.path:
            sys.path.insert(0, _p)

# problem constants (hardcoded per contract)
B, FEAT, H, W = 16, 64, 256, 256
BIN_NUM, HALF = 20, 10
N_CORES = 8
BPC = B // N_CORES                # batches per core
P = BPC * FEAT                    # 128 partitions
FREE = H * W                      # 65536 free elems per partition
NX = 4096                         # x-tile cols (DMA/copy/features/chain)
N = 2048                          # psum-half cols (PSUM: 4 banks, 2 in flight)
# 2048-col head/tail tiles shorten pipeline fill/drain; every tile is a
# whole number of 2048-col psum halves so the PSUM pool stays one size.
TILES = [N, N] + [NX] * 14 + [N, N]
assert sum(TILES) == FREE
MARGIN = 0.3                      # composite min partial slope

# --- term assignment across engines (sums to 19 terms, k=1..19) -----------
S_ACT = [6, 7, 8, 9, 10, 19]              # ACT Prelu chain (fp32; has big d10)
PAIRS = []                                # DVE PAIRT pairs: terms k and k+10
F_D = [1, 2, 3, 4, 5, 11, 12, 13, 14, 15, 16, 17, 18]  # DVE TSP feats -> PE
F_P = []                                  # Pool is ~10x slower than modeled
_used = sorted(S_ACT + [k for p in PAIRS for k in (p, p + 10)] + F_D + F_P)
assert _used == list(range(1, 20)), _used
NSRC = len(F_D) + len(F_P) + 1            # PE moving sources (+1 for h)

TK = lambda k: float((k - HALF) / 10.0)
J = len(S_ACT)

# coef column layout: alpha[J], a[J], c[J], C0, C1, then per pair (d_k, d_k+10)
NCOEF = 3 * J + 2 + 2 * len(PAIRS)

_STATE: dict = {}


def _register_ops():
    """Register the custom DVE pair op (idempotent)."""
    import concourse.dve_ops as dve_ops
    from concourse.dve_ops import DveOp
    from concourse.dve_spec import (
        C0, C1, C2, One, Spec, Src0, Src1, lower, relu, _has_src1,
    )
    from concourse.dve_uop import DveOpSpec

    if "PAIRT_MTLU" in dve_ops._SUB_OPCODE_FOR_NAME:
        by = {op.name: op for op in dve_ops.OPS}
        return by["PAIRT_MTLU"]

    def _ref_pair(in0, in1, s0, s1, imm2):
        a = in0 - imm2
        return in1 + s0 * np.maximum(a, 0) + s1 * np.maximum(a - 1.0, 0)

    def _mk(name, spec):
        row = dve_ops._CUSTOM_DVE_ROW_BASE + len(dve_ops.OPS)
        assert row < 0x20
        shas = {}
        for ver in ("v3", "v4"):
            try:
                u = lower(spec, ver=ver)
                shas[ver] = DveOpSpec(
                    name=name, opcode=row, uops=u, rd1_en=_has_src1(spec)
                ).sha(ver)
            except Exception:
                pass
        op = DveOp(name, spec, subdim=False, uops_sha=shas)
        dve_ops.OPS.append(op)
        dve_ops._SUB_OPCODE_FOR_NAME[name] = row
        dve_ops.CUSTOM_DVE_SPECS[name] = spec
        return op

    pair = _mk(
        "PAIRT_MTLU",
        Spec(
            body=Src1 + C0 * relu(Src0 - C2) + C1 * relu(Src0 - (C2 + One)),
            reference=_ref_pair,
        ),
    )
    return pair


def _build_module():
    import concourse.bacc as bacc
    import concourse.bass as bass
    import concourse.tile as tile
    from concourse import mybir

    PAIRT = _register_ops()

    nc = bacc.Bacc(
        "TRN2", target_bir_lowering=False, debug=False, num_devices=N_CORES
    )
    f32 = mybir.dt.float32
    bf16 = mybir.dt.bfloat16
    AF = mybir.ActivationFunctionType
    ALU = mybir.AluOpType
    x_in = nc.dram_tensor("x", [P, FREE], f32, kind="ExternalInput")
    coef = nc.dram_tensor("coef", [P, NCOEF], f32, kind="ExternalInput")
    diag = nc.dram_tensor("diag", [P, NSRC * 128], bf16, kind="ExternalInput")
    out = nc.dram_tensor("out", [P, FREE], f32, kind="ExternalOutput")

    NB = N // 512  # psum banks per chunk

    with tile.TileContext(nc) as tc:
        with (
            tc.tile_pool(name="coefp", bufs=1) as cpool,
            tc.tile_pool(name="xp", bufs=2) as xpool,
            tc.tile_pool(name="xbp", bufs=1) as xbpool,
            tc.tile_pool(name="hp", bufs=2) as hpool,
            tc.tile_pool(name="hbp", bufs=1) as hbpool,
            tc.tile_pool(name="fp", bufs=1) as fpool,
            tc.tile_pool(name="accp", bufs=2) as accpool,
            tc.tile_pool(name="psum", bufs=2, space="PSUM") as ppool,
        ):
            ct = cpool.tile([P, NCOEF], f32)
            nc.sync.dma_start(ct[:], coef[:])
            dt_tile = cpool.tile([P, NSRC * 128], bf16)
            nc.sync.dma_start(dt_tile[:], diag[:])

            def col(j):
                return ct[:, j : j + 1]

            c_alpha, c_a, c_c = 0, J, 2 * J
            c_C0, c_C1 = 3 * J, 3 * J + 1
            c_pair = 3 * J + 2

            off = 0
            for ci, S in enumerate(TILES):
                xsl = slice(off, off + S)
                xr = xpool.tile([P, S], f32, tag="xr")
                nc.sync.dma_start(xr[:], x_in[:, xsl])

                # bf16 copy of x feeding the TSP features
                xb = xbpool.tile([P, S], bf16, tag="xb")
                nc.vector.tensor_copy(xb[:], xr[:])

                # --- features (bf16): DVE TSP at 4x ----------------------
                feats = []
                for k in F_D:
                    f = fpool.tile([P, S], bf16, tag=f"fd{k}")
                    nc.vector.tensor_scalar(
                        f[:], xb[:], TK(k), 0.0, ALU.subtract, ALU.max
                    )
                    feats.append(f)

                # --- ACT: composed Prelu chain (fp32, bf16 final) --------
                h = xr
                for s in range(J):
                    last = s == J - 1
                    if last:
                        hn = hbpool.tile([P, S], bf16, tag="hb")
                    else:
                        hn = hpool.tile([P, S], f32, tag="h")
                    nc.scalar.activation(
                        hn[:], h[:], AF.Prelu,
                        bias=col(c_c + s),
                        scale=col(c_a + s) if last else 1.0,
                        alpha=col(c_alpha + s),
                    )
                    h = hn

                srcs = feats + [h]
                for half in range(S // N):
                    hs = slice(half * N, (half + 1) * N)
                    ps = ppool.tile([P, N], f32, tag="ps")

                    # --- PE: accumulate d_k*feat_k + h into PSUM ---------
                    for i, src in enumerate(srcs):
                        lhsT = dt_tile[:, i * 128 : (i + 1) * 128]
                        for b in range(NB):
                            bs = slice(b * 512, (b + 1) * 512)
                            ms = slice(half * N + b * 512, half * N + (b + 1) * 512)
                            nc.tensor.matmul(
                                ps[:, bs], lhsT, src[:, ms],
                                start=(i == 0), stop=(i == len(srcs) - 1),
                            )

                    # --- DVE: affine + pairs, seeded from PSUM -----------
                    acc = accpool.tile([P, N], f32, tag="acc")
                    nc.vector._custom_dve(
                        PAIRT, out=acc[:], in0=xr[:, hs], in1=ps[:],
                        s0=col(c_C0), s1=col(c_C1), imm2=-9.0,
                    )
                    for j, k in enumerate(PAIRS):
                        nxt = accpool.tile([P, N], f32, tag="acc")
                        nc.vector._custom_dve(
                            PAIRT, out=nxt[:], in0=xr[:, hs], in1=acc[:],
                            s0=col(c_pair + 2 * j), s1=col(c_pair + 2 * j + 1),
                            imm2=TK(k),
                        )
                        acc = nxt
                    nc.sync.dma_start(
                        out[:, off + half * N : off + (half + 1) * N], acc[:]
                    )
                off += S

    nc.compile()
    return nc


def _tables(mtlu_y: np.ndarray, mtlu_y_: np.ndarray):
    """Host-side coefficient + diagonal tables."""
    y = mtlu_y.astype(np.float64)
    y_ = mtlu_y_.astype(np.float64)
    index = (np.arange(BIN_NUM) - (HALF - 1)).astype(np.float64)
    w = (y - y_) / 0.1
    b = y - (y - y_) * index
    d = np.zeros((FEAT, BIN_NUM), np.float64)
    d[:, 1:] = w[:, 1:] - w[:, :-1]

    c = np.zeros((FEAT, NCOEF), np.float64)
    S = sorted(S_ACT)
    dd = d[:, S]
    sig = np.concatenate([np.zeros((FEAT, 1)), np.cumsum(dd, 1)], 1)
    lam = np.maximum(MARGIN, MARGIN - sig.min(1))
    s = lam[:, None] + sig
    alpha = s[:, :-1] / s[:, 1:]
    a = np.ones((FEAT, J))
    a[:, -1] = s[:, -1]
    T = np.array([TK(k) for k in S])
    cc_ = np.zeros((FEAT, J))
    hT = np.broadcast_to(T[None, :], (FEAT, J)).copy()
    for i in range(J):
        ci = -(a[:, i] * hT[:, i])
        cc_[:, i] = ci
        u = a[:, i : i + 1] * hT + ci[:, None]
        hT = np.where(u > 0, u, alpha[:, i : i + 1] * u)
    h0 = np.zeros((FEAT, 1))
    for i in range(J):
        u = a[:, i : i + 1] * h0 + cc_[:, i : i + 1]
        h0 = np.where(u > 0, u, alpha[:, i : i + 1] * u)
    g0 = sum(d[:, k] * max(0.0 - TK(k), 0.0) for k in S)
    Bc = h0[:, 0] - g0
    w_fix = w[:, 0] - lam
    b_fix = b[:, 0] - Bc
    # [[1,1],[9,8]]^-1: C0*relu(x+9)+C1*relu(x+8) == w_fix*x + b_fix
    c[:, 0:J] = alpha
    c[:, J : 2 * J] = a
    c[:, 2 * J : 3 * J] = cc_
    c[:, 3 * J] = b_fix - 8.0 * w_fix
    c[:, 3 * J + 1] = 9.0 * w_fix - b_fix
    for j, k in enumerate(PAIRS):
        c[:, 3 * J + 2 + 2 * j] = d[:, k]
        c[:, 3 * J + 2 + 2 * j + 1] = d[:, k + 10]
    coef = np.tile(c.astype(np.float32), (BPC, 1))  # [128, NCOEF]

    import ml_dtypes

    dp = np.tile(d, (BPC, 1))  # [128, 20]
    dg = np.zeros((P, NSRC, 128), np.float32)
    for i, k in enumerate(F_D + F_P):
        dg[np.arange(P), i, np.arange(P)] = dp[:, k]
    dg[np.arange(P), NSRC - 1, np.arange(P)] = 1.0  # h passthrough
    diag = dg.reshape(P, NSRC * 128).astype(ml_dtypes.bfloat16)
    return coef, diag


def kernel(x: np.ndarray, mtlu_y: np.ndarray, mtlu_y_: np.ndarray) -> np.ndarray:
    from concourse.bass_utils import run_bass_kernel_spmd

    if "nc" not in _STATE:
        _STATE["nc"] = _build_module()
    nc = _STATE["nc"]

    coef, diag = _tables(np.asarray(mtlu_y), np.asarray(mtlu_y_))
    xs = np.ascontiguousarray(x, dtype=np.float32).reshape(B, FEAT, FREE)
    in_maps = [
        {
            "x": xs[i * BPC : (i + 1) * BPC].reshape(P, FREE),
            "coef": coef,
            "diag": diag,
        }
        for i in range(N_CORES)
    ]
    res = run_bass_kernel_spmd(
        nc,
        in_maps,
        core_ids=list(range(N_CORES)),
        trace=bool(int(__import__("os").environ.get("MTLU_TRACE", "0"))),
    )
    _STATE["last_results"] = res
    out = np.concatenate(
        [r["out"].reshape(BPC, FEAT, H, W) for r in res.results], axis=0
    )
    return out


# revision 26
# speedup vs baseline: 1.0183x; 1.0183x over previous
"""MTLU (histogram-binning piecewise-linear unit) Trainium2 kernel, v2.

Math: out = w[c,idx]*x + b[c,idx], idx = clip(floor(x/0.1)+10, 0, 19).
Continuous PWL with uniform breakpoints t_k=(k-10)/10, k=1..19:
    out = w0[c]*x + b0[c] + sum_k d_k[c]*relu(x - t_k).

v2 spreads the 19 relu terms over FOUR engines:
  ACT   J-deep composed-Prelu chain (terms S_ACT, fp32) -> h
  DVE   TSP relu features at 4x bf16 (terms F_D) + PAIRT pair chain
        (terms PAIRS k,k+10) + the affine correction (first PAIRT,
        imm2=-9 trick: relu(x+9),relu(x+8) always active)
  Pool  TSP relu features (terms F_P, GPSIMD)
  PE    diagonal matmuls accumulate d_k*feat_k + 1*h into PSUM (bf16
        moving operands, per-channel d_k on the diagonal)
The DVE PAIRT chain is seeded with Src1 = PSUM (reads the PE total),
adds the affine + pair terms, writes the fp32 output.

Sharding: pure data parallel over batch - 2 batches/core x 8 cores.
Layout [2*64, 65536]: channel on partitions, coefficients per-partition.
"""

import sys

import numpy as np

try:  # concourse is normally on sys.path via sitecustomize
    import concourse  # noqa: F401
except ImportError:  # pragma: no cover
    for _p in ("/opt/trn_rl_repo", "/root/.axon_site/_ro/trn_rl_repo"):
        if _p not in sys.path:
            sys.path.insert(0, _p)

# problem constants (hardcoded per contract)
B, FEAT, H, W = 16, 64, 256, 256
BIN_NUM, HALF = 20, 10
N_CORES = 8
BPC = B // N_CORES                # batches per core
P = BPC * FEAT                    # 128 partitions
FREE = H * W                      # 65536 free elems per partition
NX = 4096                         # x-tile cols (DMA/copy/features/chain)
N = 2048                          # psum-half cols (PSUM: 4 banks, 2 in flight)
# 2048-col head/tail tiles shorten pipeline fill/drain; every tile is a
# whole number of 2048-col psum halves so the PSUM pool stays one size.
TILES = [N] + [NX] * 15 + [N]
assert sum(TILES) == FREE
MARGIN = 0.3                      # composite min partial slope

# --- term assignment across engines (sums to 19 terms, k=1..19) -----------
S_ACT = [6, 7, 8, 9, 10, 19]              # ACT Prelu chain (fp32; has big d10)
PAIRS = []                                # DVE PAIRT pairs: terms k and k+10
F_D = [1, 2, 3, 4, 5, 11, 12, 13, 14, 15, 16, 17, 18]  # DVE TSP feats -> PE
F_P = []                                  # Pool is ~10x slower than modeled
_used = sorted(S_ACT + [k for p in PAIRS for k in (p, p + 10)] + F_D + F_P)
assert _used == list(range(1, 20)), _used
NSRC = len(F_D) + len(F_P) + 1            # PE moving sources (+1 for h)

TK = lambda k: float((k - HALF) / 10.0)
J = len(S_ACT)

# coef column layout: alpha[J], a[J], c[J], C0, C1, then per pair (d_k, d_k+10)
NCOEF = 3 * J + 2 + 2 * len(PAIRS)

_STATE: dict = {}


def _register_ops():
    """Register the custom DVE pair op (idempotent)."""
    import concourse.dve_ops as dve_ops
    from concourse.dve_ops import DveOp
    from concourse.dve_spec import (
        C0, C1, C2, One, Spec, Src0, Src1, lower, relu, _has_src1,
    )
    from concourse.dve_uop import DveOpSpec

    if "PAIRT_MTLU" in dve_ops._SUB_OPCODE_FOR_NAME:
        by = {op.name: op for op in dve_ops.OPS}
        return by["PAIRT_MTLU"]

    def _ref_pair(in0, in1, s0, s1, imm2):
        a = in0 - imm2
        return in1 + s0 * np.maximum(a, 0) + s1 * np.maximum(a - 1.0, 0)

    def _mk(name, spec):
        row = dve_ops._CUSTOM_DVE_ROW_BASE + len(dve_ops.OPS)
        assert row < 0x20
        shas = {}
        for ver in ("v3", "v4"):
            try:
                u = lower(spec, ver=ver)
                shas[ver] = DveOpSpec(
                    name=name, opcode=row, uops=u, rd1_en=_has_src1(spec)
                ).sha(ver)
            except Exception:
                pass
        op = DveOp(name, spec, subdim=False, uops_sha=shas)
        dve_ops.OPS.append(op)
        dve_ops._SUB_OPCODE_FOR_NAME[name] = row
        dve_ops.CUSTOM_DVE_SPECS[name] = spec
        return op

    pair = _mk(
        "PAIRT_MTLU",
        Spec(
            body=Src1 + C0 * relu(Src0 - C2) + C1 * relu(Src0 - (C2 + One)),
            reference=_ref_pair,
        ),
    )
    return pair


def _build_module():
    import concourse.bacc as bacc
    import concourse.bass as bass
    import concourse.tile as tile
    from concourse import mybir

    PAIRT = _register_ops()

    nc = bacc.Bacc(
        "TRN2", target_bir_lowering=False, debug=False, num_devices=N_CORES
    )
    f32 = mybir.dt.float32
    bf16 = mybir.dt.bfloat16
    AF = mybir.ActivationFunctionType
    ALU = mybir.AluOpType
    x_in = nc.dram_tensor("x", [P, FREE], f32, kind="ExternalInput")
    coef = nc.dram_tensor("coef", [P, NCOEF], f32, kind="ExternalInput")
    diag = nc.dram_tensor("diag", [P, NSRC * 128], bf16, kind="ExternalInput")
    out = nc.dram_tensor("out", [P, FREE], f32, kind="ExternalOutput")

    NB = N // 512  # psum banks per chunk

    with tile.TileContext(nc) as tc:
        with (
            tc.tile_pool(name="coefp", bufs=1) as cpool,
            tc.tile_pool(name="xp", bufs=2) as xpool,
            tc.tile_pool(name="xbp", bufs=1) as xbpool,
            tc.tile_pool(name="hp", bufs=2) as hpool,
            tc.tile_pool(name="hbp", bufs=1) as hbpool,
            tc.tile_pool(name="fp", bufs=1) as fpool,
            tc.tile_pool(name="accp", bufs=2) as accpool,
            tc.tile_pool(name="psum", bufs=2, space="PSUM") as ppool,
        ):
            ct = cpool.tile([P, NCOEF], f32)
            nc.sync.dma_start(ct[:], coef[:])
            dt_tile = cpool.tile([P, NSRC * 128], bf16)
            nc.sync.dma_start(dt_tile[:], diag[:])

            def col(j):
                return ct[:, j : j + 1]

            c_alpha, c_a, c_c = 0, J, 2 * J
            c_C0, c_C1 = 3 * J, 3 * J + 1
            c_pair = 3 * J + 2

            off = 0
            for ci, S in enumerate(TILES):
                xsl = slice(off, off + S)
                xr = xpool.tile([P, S], f32, tag="xr")
                nc.sync.dma_start(xr[:], x_in[:, xsl])

                # bf16 copy of x feeding the TSP features
                xb = xbpool.tile([P, S], bf16, tag="xb")
                nc.vector.tensor_copy(xb[:], xr[:])

                # --- features (bf16): DVE TSP at 4x ----------------------
                feats = []
                for k in F_D:
                    f = fpool.tile([P, S], bf16, tag=f"fd{k}")
                    nc.vector.tensor_scalar(
                        f[:], xb[:], TK(k), 0.0, ALU.subtract, ALU.max
                    )
                    feats.append(f)

                # --- ACT: composed Prelu chain (fp32, bf16 final) --------
                h = xr
                for s in range(J):
                    last = s == J - 1
                    if last:
                        hn = hbpool.tile([P, S], bf16, tag="hb")
                    else:
                        hn = hpool.tile([P, S], f32, tag="h")
                    nc.scalar.activation(
                        hn[:], h[:], AF.Prelu,
                        bias=col(c_c + s),
                        scale=col(c_a + s) if last else 1.0,
                        alpha=col(c_alpha + s),
                    )
                    h = hn

                srcs = feats + [h]
                for half in range(S // N):
                    hs = slice(half * N, (half + 1) * N)
                    ps = ppool.tile([P, N], f32, tag="ps")

                    # --- PE: accumulate d_k*feat_k + h into PSUM ---------
                    for i, src in enumerate(srcs):
                        lhsT = dt_tile[:, i * 128 : (i + 1) * 128]
                        for b in range(NB):
                            bs = slice(b * 512, (b + 1) * 512)
                            ms = slice(half * N + b * 512, half * N + (b + 1) * 512)
                            nc.tensor.matmul(
                                ps[:, bs], lhsT, src[:, ms],
                                start=(i == 0), stop=(i == len(srcs) - 1),
                            )

                    # --- DVE: affine + pairs, seeded from PSUM -----------
                    acc = accpool.tile([P, N], f32, tag="acc")
                    nc.vector._custom_dve(
                        PAIRT, out=acc[:], in0=xr[:, hs], in1=ps[:],
                        s0=col(c_C0), s1=col(c_C1), imm2=-9.0,
                    )
                    for j, k in enumerate(PAIRS):
                        nxt = accpool.tile([P, N], f32, tag="acc")
                        nc.vector._custom_dve(
                            PAIRT, out=nxt[:], in0=xr[:, hs], in1=acc[:],
                            s0=col(c_pair + 2 * j), s1=col(c_pair + 2 * j + 1),
                            imm2=TK(k),
                        )
                        acc = nxt
                    nc.sync.dma_start(
                        out[:, off + half * N : off + (half + 1) * N], acc[:]
                    )
                off += S

    nc.compile()
    return nc


def _tables(mtlu_y: np.ndarray, mtlu_y_: np.ndarray):
    """Host-side coefficient + diagonal tables."""
    y = mtlu_y.astype(np.float64)
    y_ = mtlu_y_.astype(np.float64)
    index = (np.arange(BIN_NUM) - (HALF - 1)).astype(np.float64)
    w = (y - y_) / 0.1
    b = y - (y - y_) * index
    d = np.zeros((FEAT, BIN_NUM), np.float64)
    d[:, 1:] = w[:, 1:] - w[:, :-1]

    c = np.zeros((FEAT, NCOEF), np.float64)
    S = sorted(S_ACT)
    dd = d[:, S]
    sig = np.concatenate([np.zeros((FEAT, 1)), np.cumsum(dd, 1)], 1)
    lam = np.maximum(MARGIN, MARGIN - sig.min(1))
    s = lam[:, None] + sig
    alpha = s[:, :-1] / s[:, 1:]
    a = np.ones((FEAT, J))
    a[:, -1] = s[:, -1]
    T = np.array([TK(k) for k in S])
    cc_ = np.zeros((FEAT, J))
    hT = np.broadcast_to(T[None, :], (FEAT, J)).copy()
    for i in range(J):
        ci = -(a[:, i] * hT[:, i])
        cc_[:, i] = ci
        u = a[:, i : i + 1] * hT + ci[:, None]
        hT = np.where(u > 0, u, alpha[:, i : i + 1] * u)
    h0 = np.zeros((FEAT, 1))
    for i in range(J):
        u = a[:, i : i + 1] * h0 + cc_[:, i : i + 1]
        h0 = np.where(u > 0, u, alpha[:, i : i + 1] * u)
    g0 = sum(d[:, k] * max(0.0 - TK(k), 0.0) for k in S)
    Bc = h0[:, 0] - g0
    w_fix = w[:, 0] - lam
    b_fix = b[:, 0] - Bc
    # [[1,1],[9,8]]^-1: C0*relu(x+9)+C1*relu(x+8) == w_fix*x + b_fix
    c[:, 0:J] = alpha
    c[:, J : 2 * J] = a
    c[:, 2 * J : 3 * J] = cc_
    c[:, 3 * J] = b_fix - 8.0 * w_fix
    c[:, 3 * J + 1] = 9.0 * w_fix - b_fix
    for j, k in enumerate(PAIRS):
        c[:, 3 * J + 2 + 2 * j] = d[:, k]
        c[:, 3 * J + 2 + 2 * j + 1] = d[:, k + 10]
    coef = np.tile(c.astype(np.float32), (BPC, 1))  # [128, NCOEF]

    import ml_dtypes

    dp = np.tile(d, (BPC, 1))  # [128, 20]
    dg = np.zeros((P, NSRC, 128), np.float32)
    for i, k in enumerate(F_D + F_P):
        dg[np.arange(P), i, np.arange(P)] = dp[:, k]
    dg[np.arange(P), NSRC - 1, np.arange(P)] = 1.0  # h passthrough
    diag = dg.reshape(P, NSRC * 128).astype(ml_dtypes.bfloat16)
    return coef, diag


def kernel(x: np.ndarray, mtlu_y: np.ndarray, mtlu_y_: np.ndarray) -> np.ndarray:
    from concourse.bass_utils import run_bass_kernel_spmd

    if "nc" not in _STATE:
        _STATE["nc"] = _build_module()
    nc = _STATE["nc"]

    coef, diag = _tables(np.asarray(mtlu_y), np.asarray(mtlu_y_))
    xs = np.ascontiguousarray(x, dtype=np.float32).reshape(B, FEAT, FREE)
    in_maps = [
        {
            "x": xs[i * BPC : (i + 1) * BPC].reshape(P, FREE),
            "coef": coef,
            "diag": diag,
        }
        for i in range(N_CORES)
    ]
    res = run_bass_kernel_spmd(
        nc,
        in_maps,
        core_ids=list(range(N_CORES)),
        trace=bool(int(__import__("os").environ.get("MTLU_TRACE", "0"))),
    )
    _STATE["last_results"] = res
    out = np.concatenate(
        [r["out"].reshape(BPC, FEAT, H, W) for r in res.results], axis=0
    )
    return out
